# revision 33
# baseline (speedup 1.0000x reference)
"""Trainium2 Bass kernel for nn_Block (dense transformer block), v3.

Shapes (hardcoded): x [8, 1024, 768], 12 heads x 64 head_dim, MLP hidden 16.
Sharding: data-parallel over batch, one batch element per NeuronCore (8 cores).

Design (see v2 notes):
- LN1 on host; h1/w_qkv ship as fp8(e4m3), weights pre-scaled x16.
- qkv chains, S (zero-slot) and P@V run as fp8 DoubleRow (0.5 cycles/row).
- Token-half phasing: phase A = query half 0, phase B = half 1. The entire
  half-A epilogue (proj, LN2, fc1, gelu, fc2, out DMA) interleaves into
  phase B as PE filler.
- No activation-table reloads: only the exp_and_others table is used.
  LN2 rstd = exp(-0.5*ln(var+eps)) with ln via the int-bitcast trick;
  GELU uses the tanh approximation (tanh lives in the exp table).
"""

import math
import sys

for _p in ("/root/.axon_site", "/root/.axon_site/_ro/trn_rl_repo",
           "/root/.axon_site/_ro/pypackages", "/opt/trn_rl_repo"):
    if _p not in sys.path:
        sys.path.append(_p)

import numpy as np

import concourse.bacc as bacc
import concourse.tile as tile
import concourse.mybir as mybir
from concourse.bass_utils import run_bass_kernel_spmd

FP32 = mybir.dt.float32
FP32R = mybir.dt.float32r
BF16 = mybir.dt.bfloat16
FP8 = mybir.dt.float8e4
INT32 = mybir.dt.int32
AF = mybir.ActivationFunctionType
ALU = mybir.AluOpType
DR = mybir.MatmulPerfMode.DoubleRow

N_CORES = 8
D = 768
P = 1024
H = 12
HD = 64
DT = D // 128
TT = P // 128
MLP = 16
EPS = 1e-5
SCALE = HD ** -0.5
WS = 16.0        # fp8 weight pre-scale

# rstd = exp(scale * bitcast_i32(var+eps) + bias) ~= (var+eps)^-0.5
RS_SCALE = -0.5 * math.log(2.0) / 2.0 ** 23
RS_BIAS = 0.5 * (127.0 - 0.0430357) * math.log(2.0)
GC1 = math.sqrt(2.0 / math.pi)           # gelu tanh c1
GC2 = GC1 * 0.044715                     # gelu tanh c2


def _emit(nc, tc, io):
    dma2 = [nc.sync, nc.scalar]

    with nc.allow_low_precision(reason="fp8/bf16 rounding fits error budget"), \
         tc.tile_pool(name="pers", bufs=1) as pers, \
         tc.tile_pool(name="psum", bufs=1, space="PSUM") as psum:

        with tc.tile_pool(name="wp", bufs=3) as wp, \
             tc.tile_pool(name="attn", bufs=1) as attn, \
             tc.tile_pool(name="outp", bufs=4) as outp:

            # ---------- critical-path DMAs ----------
            wqk = {}

            def load_wqk(m, eng):
                t = wp.tile([128, DT, 128], FP8, tag="wqk", name="wqk")
                eng.dma_start(
                    t[:], io["w_qk"][m].rearrange("p (o c) -> p o c", c=128))
                wqk[m] = t

            load_wqk(0, nc.sync)
            load_wqk(6, nc.scalar)
            h18 = attn.tile([128, DT, P], FP8, tag="h18", name="h18")
            nc.sync.dma_start(
                h18[:], io["h1t"].rearrange("(o p) c -> p o c", p=128))
            wv = attn.tile([128, DT, D], FP8, tag="wv", name="wv")
            nc.scalar.dma_start(
                wv[:], io["w_v"].rearrange("p (o c) -> p o c", c=D))

            # ---------- packed constants (one DMA) ----------
            consts = pers.tile([128, 48], FP32, tag="consts", name="consts")
            nc.sync.dma_start(consts[:], io["consts"][:])
            b_qk = consts[:, 0:12]
            g2_col = consts[:, 12:18]
            b_proj = consts[:, 18:24]
            b_fc2 = consts[:, 24:30]
            b_fc1 = consts[0:MLP, 31:32]
            constsr = pers.tile([128, 18], FP32R, tag="constsr",
                                name="constsr")
            nc.scalar.dma_start(constsr[:], io["constsr"][:])
            ones128 = constsr[:, 0:1]
            wg_fc1 = constsr[0:1, 2:2 + MLP]

            rsb = pers.tile([1, 1], FP32, tag="rsb", name="rsb")
            nc.vector.memset(rsb[:], RS_BIAS)
            qkz = pers.tile([128, 13, 1024], FP8, tag="qkz", name="qkz")
            nc.vector.memset(qkz[:, 12, :], 0.0)
            v8 = pers.tile([128, TT, H, 128], FP8, tag="v8", name="v8")
            nc.gpsimd.memset(v8[:, :, :, HD + 1:], 0.0)
            nc.vector.memset(v8[:, :, :, HD:HD + 1], 1.0)

            o_sb = [pers.tile([128, P], BF16, tag=f"osb{i}", name=f"osb{i}")
                    for i in range(DT)]
            out1 = [pers.tile([128, P], FP32R, tag=f"out1{i}", name=f"out1{i}")
                    for i in range(DT)]
            xt6 = pers.tile([128, DT, P], FP32R, tag="xt6", name="xt6")
            xt = [xt6[:, i, :] for i in range(DT)]
            xg2 = [pers.tile([128, P], BF16, tag=f"xg2{i}", name=f"xg2{i}")
                   for i in range(DT)]
            negmu2 = pers.tile([1, 1024], FP32R, tag="negmu2", name="negmu2")
            var2 = pers.tile([1, 1024], FP32, tag="var2", name="var2")
            rstd2 = pers.tile([1, 1024], FP32, tag="rstd2", name="rstd2")
            rstd2_bc = pers.tile([MLP, P], FP32, tag="rstd2_bc",
                                 name="rstd2_bc")
            g_raw = pers.tile([MLP, 1024], FP32, tag="g_raw", name="g_raw")
            gpre = pers.tile([MLP, P], FP32, tag="gpre", name="gpre")
            gact = pers.tile([MLP, P], BF16, tag="gact", name="gact")
            wf1 = pers.tile([128, DT, MLP], BF16, tag="wfc1", name="wfc1")
            wf2 = pers.tile([MLP, D], BF16, tag="wfc2", name="wfc2")
            wpj = {}

            with tc.tile_pool(name="ep", bufs=24) as ep, \
                 tc.tile_pool(name="sqp", bufs=2) as sqp, \
                 tc.tile_pool(name="bcp", bufs=2) as bcp, \
                 tc.tile_pool(name="recp", bufs=2) as recp, \
                 tc.tile_pool(name="glp", bufs=1) as glp:

                def chain_half(m, hs):
                    cs = slice(hs * 512, hs * 512 + 512)
                    ps = psum.tile([128, 512], FP32, tag="c", bufs=2,
                                   name="cps")
                    for d in range(3):
                        nc.tensor.matmul(ps[:], wqk[m][:, 2 * d:2 * d + 2, :],
                                         h18[:, 2 * d:2 * d + 2, cs],
                                         start=(d == 0), stop=(d == 2),
                                         perf_mode=DR)
                    nc.vector.tensor_scalar(qkz[:, m, cs], ps[:],
                                            b_qk[:, m:m + 1], None,
                                            op0=ALU.add)

                def v_half(t, half):
                    tsl = slice(t * 128, (t + 1) * 128)
                    n = 512 if half == 0 else 256
                    fs = slice(0, 512) if half == 0 else slice(512, 768)
                    hsl = slice(0, 8) if half == 0 else slice(8, 12)
                    ps = psum.tile([128, n], FP32, tag="c", bufs=2, name="vps")
                    for d in range(3):
                        nc.tensor.matmul(ps[:], h18[:, 2 * d:2 * d + 2, tsl],
                                         wv[:, 2 * d:2 * d + 2, fs],
                                         start=(d == 0), stop=(d == 2),
                                         perf_mode=DR)
                    nc.vector.tensor_scalar(
                        v8[:, t, hsl, 0:HD],
                        ps[:].rearrange("p (h d) -> p h d", d=HD),
                        1.0, None, op0=ALU.mult)

                wpjt = pers.tile([128, DT, DT, 128], BF16, tag="wpjt",
                                 name="wpjt")

                def load_wpj():
                    nc.scalar.dma_start(
                        wpjt[:],
                        io["w_proj"].rearrange("m p (o c) -> p m o c", c=128))

                def proj_m(m, hs, tag="c"):
                    cs = slice(hs * 512, hs * 512 + 512)
                    ps = psum.tile([128, 512], FP32, tag=tag, bufs=2,
                                   name="pps")
                    for dt in range(DT):
                        nc.tensor.matmul(ps[:], wpjt[:, m, dt, :],
                                         o_sb[dt][:, cs],
                                         start=(dt == 0), stop=(dt == DT - 1))
                    nc.vector.scalar_tensor_tensor(
                        out1[m][:, cs], ps[:], b_proj[:, m:m + 1],
                        xt6[:, m, cs], op0=ALU.add, op1=ALU.add)

                def stats_sums(hs, sq_engs):
                    cs = slice(hs * 512, hs * 512 + 512)
                    sum_ps = psum.tile([1, 512], FP32, tag="c", bufs=2,
                                       name="s1")
                    sum2_ps = psum.tile([1, 512], FP32, tag="c", bufs=2,
                                        name="s2")
                    for dt in range(DT):
                        sq = sqp.tile([128, 512], FP32R, tag="sq", name="sq")
                        sq_engs[dt % len(sq_engs)].tensor_mul(
                            sq[:], out1[dt][:, cs], out1[dt][:, cs])
                        nc.tensor.matmul(sum_ps[:], ones128,
                                         out1[dt][:, cs],
                                         start=(dt == 0), stop=(dt == DT - 1))
                        nc.tensor.matmul(sum2_ps[:], ones128[:], sq[:],
                                         start=(dt == 0), stop=(dt == DT - 1))
                    m2 = recp.tile([1, 512], FP32, tag="m2", name="m2")
                    nc.vector.tensor_scalar(negmu2[:, cs], sum_ps[:],
                                            -1.0 / D, None, op0=ALU.mult)
                    nc.vector.tensor_scalar(m2[:], sum2_ps[:], 1.0 / D, None,
                                            op0=ALU.mult)
                    tmp = recp.tile([1, 512], FP32, tag="tmp", name="tmp")
                    nc.vector.tensor_mul(tmp[:], negmu2[:, cs], negmu2[:, cs])
                    # var2 = (m2 + eps) - mu^2
                    nc.vector.scalar_tensor_tensor(var2[:, cs], m2[:], EPS,
                                                   tmp[:], op0=ALU.add,
                                                   op1=ALU.subtract)

                def xg2_half(hs, engs):
                    cs = slice(hs * 512, hs * 512 + 512)
                    for dt in range(DT):
                        engs[dt % len(engs)].tensor_scalar(
                            xg2[dt][:, cs], out1[dt][:, cs],
                            g2_col[:, dt:dt + 1], None, op0=ALU.mult)

                def fc1_bulk(hs):
                    cs = slice(hs * 512, hs * 512 + 512)
                    g_ps = psum.tile([MLP, 512], FP32, tag="c", bufs=2,
                                     name="gps")
                    for dt in range(DT):
                        nc.tensor.matmul(g_ps[:], wf1[:, dt, :],
                                         xg2[dt][:, cs],
                                         start=(dt == 0), stop=False)
                    nc.tensor.matmul(g_ps[:], wg_fc1,
                                     negmu2[0:1, cs], start=False, stop=True)
                    nc.vector.tensor_scalar(g_raw[:, cs], g_ps[:], 1.0, None,
                                            op0=ALU.mult)

                def mlp_head(hs):
                    """rstd + gpre + tanh-gelu for token half hs."""
                    cs = slice(hs * 512, hs * 512 + 512)
                    nc.scalar.activation(rstd2[:, cs],
                                         var2[:, cs].bitcast(INT32),
                                         AF.Exp, scale=RS_SCALE,
                                         bias=rsb[:])
                    nc.gpsimd.partition_broadcast(rstd2_bc[:, cs],
                                                  rstd2[:, cs])
                    gp = gpre[:, cs]
                    nc.vector.tensor_mul(gp, g_raw[:, cs], rstd2_bc[:, cs])
                    nc.vector.tensor_scalar(gp, gp, b_fc1, None,
                                            op0=ALU.add)
                    ta = glp.tile([MLP, 512], FP32, tag="ga", name="ga")
                    tb = glp.tile([MLP, 512], FP32, tag="gb", name="gb")
                    nc.vector.tensor_mul(ta[:], gp, gp)              # g^2
                    nc.vector.scalar_tensor_tensor(tb[:], ta[:], GC2, gp,
                                                   op0=ALU.mult,
                                                   op1=ALU.mult)    # c2 g^3
                    nc.vector.scalar_tensor_tensor(ta[:], gp, GC1, tb[:],
                                                   op0=ALU.mult,
                                                   op1=ALU.add)     # u
                    nc.scalar.activation(tb[:], ta[:], AF.Tanh)
                    nc.vector.scalar_tensor_tensor(ta[:], tb[:], 0.5, gp,
                                                   op0=ALU.mult,
                                                   op1=ALU.mult)    # .5Tg
                    nc.vector.scalar_tensor_tensor(gact[:, cs], gp, 0.5,
                                                   ta[:], op0=ALU.mult,
                                                   op1=ALU.add)

                def fc2_half(m, hs):
                    cs = slice(hs * 512, hs * 512 + 512)
                    ps = psum.tile([128, 512], FP32,
                                   tag=("c", "op")[(2 * m + hs) % 2], bufs=2,
                                   name="fps")
                    ot = outp.tile([128, 512], FP32, tag="outT", name="outT")
                    nc.tensor.matmul(ps[:], wf2[:, m * 128:(m + 1) * 128],
                                     gact[:, cs], start=True, stop=True)
                    nc.vector.scalar_tensor_tensor(ot[:], ps[:],
                                                   b_fc2[:, m:m + 1],
                                                   out1[m][:, cs],
                                                   op0=ALU.add, op1=ALU.add)
                    dma2[(2 * m + hs) % 2].dma_start(
                        io["out"][m * 128:(m + 1) * 128, cs], ot[:])

                e_tiles = {}

                def emit_S(h, hs, crange):
                    hp = h // 2
                    pp = slice((h % 2) * 64, (h % 2) * 64 + 64)
                    mq, mk = hp, 6 + hp
                    cs = slice(hs * 512, hs * 512 + 512)
                    tiles = e_tiles.setdefault((h, hs), [])
                    for c in crange:
                        sps = psum.tile([128, 2, 512], FP32, tag="sp", bufs=2,
                                        name="sps")
                        for s in range(2):
                            j = 2 * c + s
                            jsl = slice(j * 128, (j + 1) * 128)
                            nc.tensor.matmul(
                                sps[:, s, :],
                                qkz[pp, mk::(12 - mk), jsl],
                                qkz[pp, mq::(12 - mq), cs],
                                start=True, stop=True, perf_mode=DR)
                        e2 = ep.tile([128, 2, 512], FP8, tag="e2", name="e2")
                        nc.scalar.activation(e2[:], sps[:], AF.Exp,
                                             scale=SCALE / (WS * WS))
                        tiles.append(e2)

                def emit_PV(h, hs):
                    hp = h // 2
                    pp = slice((h % 2) * 64, (h % 2) * 64 + 64)
                    cs = slice(hs * 512, hs * 512 + 512)
                    tiles = e_tiles.pop((h, hs))
                    ops = psum.tile([128, 512], FP32, tag="op", bufs=2,
                                    name="ops")
                    for c in range(4):
                        nc.tensor.matmul(ops[:],
                                         v8[:, 2 * c:2 * c + 2, h, :],
                                         tiles[c][:],
                                         start=(c == 0), stop=(c == 3),
                                         perf_mode=DR)
                    rec = recp.tile([1, 512], FP32R, tag="rec", name="rec")
                    nc.vector.reciprocal(rec[:], ops[HD:HD + 1, :])
                    bc = bcp.tile([64, 512], FP32R, tag="bc", name="bc")
                    nc.gpsimd.partition_broadcast(bc[:], rec[:])
                    nc.vector.tensor_mul(o_sb[hp][pp, cs], ops[0:HD, :],
                                         bc[:])

                def wload(m):
                    return (load_wqk, m, dma2[m % 2])

                fillerA = {
                    1: [wload(1), wload(7), (chain_half, 1, 0),
                        (chain_half, 7, 0), (chain_half, 7, 1),
                        (chain_half, 1, 1)],
                    2: [(v_half, 0, 0), (v_half, 0, 1), (v_half, 1, 0),
                        (v_half, 1, 1), (v_half, 2, 0), (v_half, 2, 1)],
                    3: [wload(2), wload(8), (chain_half, 2, 0),
                        (chain_half, 8, 0), (chain_half, 8, 1),
                        (chain_half, 2, 1), (v_half, 3, 0), (v_half, 3, 1),
                        (v_half, 4, 0), (v_half, 4, 1), (v_half, 5, 0),
                        (v_half, 5, 1), (v_half, 6, 0), (v_half, 6, 1),
                        (v_half, 7, 0), (v_half, 7, 1)],
                    4: [wload(3), wload(9), (chain_half, 3, 0),
                        (chain_half, 9, 0), (chain_half, 9, 1),
                        (chain_half, 3, 1)],
                    5: [wload(4), wload(10), (chain_half, 4, 0),
                        (chain_half, 10, 0), (chain_half, 10, 1),
                        (chain_half, 4, 1)],
                    6: [wload(5), wload(11), (chain_half, 5, 0),
                        (chain_half, 11, 0), (chain_half, 11, 1),
                        (chain_half, 5, 1)],
                    7: [("xt", 0)],
                    8: [("xt", 1), ("wf", 0)],
                    9: [(load_wpj,)],
                }
                fillerB = {
                    0: [(proj_m, 0, 0), (proj_m, 1, 0)],
                    1: [(proj_m, 2, 0), (proj_m, 3, 0)],
                    2: [(proj_m, 4, 0), (proj_m, 5, 0)],
                    3: [(stats_sums, 0, (nc.gpsimd,))],
                    4: [(xg2_half, 0, (nc.gpsimd, nc.vector))],
                    5: [(fc1_bulk, 0)],
                    6: [(mlp_head, 0)],
                    8: [(fc2_half, 0, 0), (fc2_half, 1, 0),
                        (fc2_half, 2, 0)],
                    9: [(fc2_half, 3, 0), (fc2_half, 4, 0),
                        (fc2_half, 5, 0)],
                }

                def run_filler(table, h):
                    for item in table.pop(h, []):
                        if item[0] == "xt":
                            hh = item[1]
                            dma2[hh].dma_start(
                                xt6[:, 3 * hh:3 * hh + 3, :],
                                io["xt"][384 * hh:384 * hh + 384, :]
                                .rearrange("(o p) c -> p o c", p=128))
                        elif item[0] == "wf":
                            nc.scalar.dma_start(
                                wf1[:],
                                io["w_fc1"].rearrange("p (o c) -> p o c",
                                                      c=MLP))
                            nc.sync.dma_start(wf2[:], io["w_fc2"][:])
                        else:
                            item[0](*item[1:])

                # ---- opening: minimal deps before the first exp ----
                chain_half(0, 0)
                chain_half(6, 0)
                emit_S(0, 0, range(0, 2))
                chain_half(6, 1)
                chain_half(0, 1)
                emit_S(0, 0, range(2, 4))

                # ---- phase A ----
                LAG_A = 3
                for h in range(1, H):
                    emit_S(h, 0, range(4))
                    run_filler(fillerA, h)
                    if h - LAG_A >= 0:
                        emit_PV(h - LAG_A, 0)
                emit_PV(9, 0)
                emit_PV(10, 0)
                # ---- phase B (epilogue A as filler) ----
                emit_S(0, 1, range(4))
                emit_PV(11, 0)
                run_filler(fillerB, 0)
                LAG_B = 2
                for h in range(1, H):
                    emit_S(h, 1, range(4))
                    run_filler(fillerB, h)
                    if h - LAG_B >= 0:
                        emit_PV(h - LAG_B, 1)
                for h in range(H - LAG_B, H):
                    emit_PV(h, 1)

                # ---- tail: half-B epilogue (stats chase proj per-m) ----
                cs1 = slice(512, 1024)
                sum_ps = psum.tile([1, 512], FP32, tag="c", bufs=2, name="s1")
                sum2_ps = psum.tile([1, 512], FP32, tag="c", bufs=2,
                                    name="s2")
                sq_engs = (nc.vector, nc.gpsimd)
                for m in range(DT):
                    proj_m(m, 1, tag="op")
                    sq = sqp.tile([128, 512], FP32R, tag="sq", name="sq")
                    sq_engs[m % 2].tensor_mul(sq[:], out1[m][:, cs1],
                                              out1[m][:, cs1])
                    nc.tensor.matmul(sum_ps[:], ones128, out1[m][:, cs1],
                                     start=(m == 0), stop=(m == DT - 1))
                    nc.tensor.matmul(sum2_ps[:], ones128, sq[:],
                                     start=(m == 0), stop=(m == DT - 1))
                    nc.gpsimd.tensor_scalar(xg2[m][:, cs1], out1[m][:, cs1],
                                            g2_col[:, m:m + 1], None,
                                            op0=ALU.mult)
                m2 = recp.tile([1, 512], FP32, tag="m2", name="m2")
                nc.scalar.mul(negmu2[:, cs1], sum_ps[:], -1.0 / D)
                nc.scalar.mul(m2[:], sum2_ps[:], 1.0 / D)
                tmp = recp.tile([1, 512], FP32, tag="tmp", name="tmp")
                nc.vector.tensor_mul(tmp[:], negmu2[:, cs1], negmu2[:, cs1])
                nc.vector.scalar_tensor_tensor(var2[:, cs1], m2[:], EPS,
                                               tmp[:], op0=ALU.add,
                                               op1=ALU.subtract)
                fc1_bulk(1)
                # rstd + one-shot gelu (table load hides in the idle tail)
                nc.scalar.activation(rstd2[:, cs1],
                                     var2[:, cs1].bitcast(INT32),
                                     AF.Exp, scale=RS_SCALE, bias=rsb[:])
                nc.gpsimd.partition_broadcast(rstd2_bc[:, cs1],
                                              rstd2[:, cs1])
                gp = gpre[:, cs1]
                nc.vector.tensor_mul(gp, g_raw[:, cs1], rstd2_bc[:, cs1])
                nc.scalar.activation(gact[:, cs1], gp, AF.Gelu_apprx_tanh,
                                     bias=b_fc1)
                for m in range(DT):
                    fc2_half(m, 1)


def build():
    nc = bacc.Bacc("TRN2", target_bir_lowering=False, debug=False,
                   num_devices=N_CORES)
    io = {
        "h1t": nc.dram_tensor("h1t", [D, P], FP8, kind="ExternalInput").ap(),
        "xt": nc.dram_tensor("xt", [D, P], FP32R, kind="ExternalInput").ap(),
        "w_qk": nc.dram_tensor("w_qk", [12, 128, DT * 128], FP8,
                               kind="ExternalInput").ap(),
        "consts": nc.dram_tensor("consts", [128, 48], FP32,
                                 kind="ExternalInput").ap(),
        "constsr": nc.dram_tensor("constsr", [128, 18], FP32R,
                                  kind="ExternalInput").ap(),
        "w_v": nc.dram_tensor("w_v", [128, DT * D], FP8,
                              kind="ExternalInput").ap(),
        "w_proj": nc.dram_tensor("w_proj", [DT, 128, DT * 128], BF16,
                                 kind="ExternalInput").ap(),
        "w_fc1": nc.dram_tensor("w_fc1", [128, DT * MLP], BF16,
                                kind="ExternalInput").ap(),
        "w_fc2": nc.dram_tensor("w_fc2", [MLP, D], BF16,
                                kind="ExternalInput").ap(),
        "out": nc.dram_tensor("out", [D, P], FP32, kind="ExternalOutput").ap(),
    }
    with tile.TileContext(nc) as tc:
        _emit(nc, tc, io)
    nc.compile()
    return nc


def prep_inputs(x, g1, b1, w_qkv, b_qkv, w_proj, b_proj, g2, b2,
                w_fc1, b_fc1, w_fc2, b_fc2):
    """Host-side re-layout of the full inputs into per-core in_maps."""
    import ml_dtypes
    f32 = np.float32
    f8 = mybir.dt.np(FP8)
    asf = lambda a: np.ascontiguousarray(a, dtype=f32)
    asb = lambda a: np.ascontiguousarray(np.asarray(a, dtype=np.float64),
                                         dtype=ml_dtypes.bfloat16)
    as8 = lambda a: np.ascontiguousarray(np.asarray(a, dtype=f32), dtype=f8)

    i3, d, h = np.meshgrid(np.arange(3), np.arange(HD), np.arange(H),
                           indexing="ij")
    perm = (i3 * D + d * H + h).reshape(3, HD, H).transpose(0, 2, 1).reshape(-1)
    w_re = np.asarray(w_qkv, dtype=f32)[:, perm].astype(np.float64)
    b_re = np.asarray(b_qkv, dtype=f32)[perm].astype(np.float64)

    w_proj = np.asarray(w_proj, dtype=f32).astype(np.float64)
    g1 = np.asarray(g1, f32).astype(np.float64)
    b1 = np.asarray(b1, f32).astype(np.float64)
    g2 = np.asarray(g2, f32); b2 = np.asarray(b2, f32)
    w_fc1 = np.asarray(w_fc1, f32); w_fc2 = np.asarray(w_fc2, f32)
    w_qk = w_re[:, :2 * D]
    w_v = w_re[:, 2 * D:]
    b_qk = b_re[:2 * D]
    b_v = b_re[2 * D:]
    b_proj_eff = np.asarray(b_proj, dtype=f32) + (b_v @ w_proj).astype(f32)

    consts = np.zeros((128, 48), f32)
    consts[:, 0:12] = (b_qk * WS).reshape(12, 128).T
    consts[:, 12:18] = g2.reshape(6, 128).T
    consts[:, 18:24] = b_proj_eff.reshape(6, 128).T
    consts[:, 24:30] = np.asarray(b_fc2, f32).reshape(6, 128).T
    consts[0:MLP, 31] = np.asarray(b_fc1, f32) + w_fc1.T @ b2
    constsr = np.zeros((128, 18), f32)
    constsr[:, 0] = 1.0
    constsr[0, 2:2 + MLP] = w_fc1.T @ g2
    common = {
        "consts": consts,
        "constsr": constsr,
        "w_qk": as8((w_qk * WS).reshape(6, 128, 12, 128).transpose(2, 1, 0, 3)
                    .reshape(12, 128, 768)),
        "w_v": as8((w_v * WS).reshape(6, 128, D).transpose(1, 0, 2)
                   .reshape(128, 6 * D)),
        "w_proj": asb((w_proj / WS).reshape(6, 128, 6, 128)
                      .transpose(2, 1, 0, 3).reshape(6, 128, 768)),
        "w_fc1": asb(w_fc1.reshape(6, 128, MLP).transpose(1, 0, 2)
                     .reshape(128, 6 * MLP)),
        "w_fc2": asb(w_fc2),
    }
    x = np.asarray(x, dtype=f32)
    maps = []
    for i in range(N_CORES):
        xi = x[i].astype(np.float64)                       # [P, D]
        mu = xi.mean(axis=1, keepdims=True)
        var = ((xi - mu) ** 2).mean(axis=1, keepdims=True)
        h1 = (xi - mu) / np.sqrt(var + EPS) * g1 + b1      # [P, D]
        maps.append(dict(common, xt=asf(x[i].T), h1t=as8(h1.T)))
    return maps


_NC_CACHE = None


def kernel(**inputs):
    global _NC_CACHE
    if _NC_CACHE is None:
        _NC_CACHE = build()
    in_maps = prep_inputs(**inputs)
    res = run_bass_kernel_spmd(_NC_CACHE, in_maps, list(range(N_CORES)))
    return np.stack([res.results[i]["out"].T for i in range(N_CORES)])
